# revision 3
# baseline (speedup 1.0000x reference)
"""Trainium2 Bass kernel for nn_ApproachingMomentumLoss (8 NeuronCores, data parallel).

Math: the reference clamps gt_distance at RADIUS=20 == DECAY_START, so momentum
is identically 1.0 in the forward pass and the loss reduces to
    loss = sum_r sum_i |cumsum(v*m)_ri - D_ri| * m_ri / (max_i D_ri + 1e-6)
           / (sum(m) + 1e-6)
with D = min(distance to nearest (boundary | ~mask), 20), virtual boundaries at
-1 and T.

Distribution: pure data parallel, one row of T=4096 per core as [128 x 32]
(element i = p*32 + f). The 20-clamp makes the distance transform local, so the
host ships 20-element halo'd uint8 views and each partition computes its
distances with two tensor_tensor_scan min-plus scans and NO cross-partition
carries. The only cross-partition work is the cumsum carry (strict-lower
triangular matmul) and the final reductions (matmul with ones / identity).
Each core returns partial [loss_row/scale_row, mask_sum]; the unshard step sums
the 8 partials and divides.
"""
import numpy as np
import concourse.bass as bass
import concourse.bacc as bacc
import concourse.mybir as mybir
import concourse.tile as tile
from concourse.bass_utils import run_bass_kernel_spmd

f32 = mybir.dt.float32
i32 = mybir.dt.int32
u8 = mybir.dt.uint8
AL = mybir.AluOpType
AX = mybir.AxisListType

BIG = 1.0e9
N_CORES = 8
P, F, H = 128, 32, 20
W = F + H  # 52

_NC = None


def _build():
    nc = bacc.Bacc("TRN2", target_bir_lowering=False, debug=False, num_devices=N_CORES)
    bm_ext = nc.dram_tensor("bm", [P, 4 * W], u8, kind="ExternalInput")
    v_ext = nc.dram_tensor("v", [P, F], f32, kind="ExternalInput")
    out_ext = nc.dram_tensor("out", [1, 2], f32, kind="ExternalOutput")

    with tile.TileContext(nc) as tc:
        with (
            tc.tile_pool(name="sb", bufs=1) as pool,
            tc.tile_pool(name="ps", bufs=1, space="PSUM") as psum,
        ):
            # inputs on two sequencers -> parallel HWDGE queues
            bmT = pool.tile([P, 4 * W], u8)
            nc.sync.dma_start(bmT[:], bm_ext.ap())
            vT = pool.tile([P, F], f32)
            nc.scalar.dma_start(vT[:], v_ext.ap())

            # constants (hoisted pre-barrier by _hoist_prologue)
            ones52 = pool.tile([P, W], f32)
            nc.vector.memset(ones52, 1.0)
            ones128 = pool.tile([P, 1], f32)
            nc.vector.memset(ones128, 1.0)
            io128 = pool.tile([P, P], i32)
            nc.gpsimd.iota(io128, pattern=[[1, P]], base=0, channel_multiplier=-1)  # c - p
            LT = pool.tile([P, P], f32)
            nc.vector.tensor_scalar(LT, io128, 0, None, AL.is_gt)   # strict lower (as lhsT)
            I128 = pool.tile([P, P], f32)
            nc.vector.tensor_scalar(I128, io128, 0, None, AL.is_equal)

            # cost tiles: C = BIG*(1-b)*m for both halo'd views
            NB = pool.tile([P, 2 * W], f32)
            nc.vector.tensor_scalar(NB, bmT[:, 0:2 * W], -BIG, BIG, AL.mult, AL.add)
            MF2 = pool.tile([P, 2 * W], f32)
            nc.vector.tensor_scalar(MF2, bmT[:, 2 * W:4 * W], 1, None, AL.mult)
            MF = MF2[:, W:W + F]  # m_R cols 0:32 == the raw row mask
            vm = pool.tile([P, F], f32)
            nc.vector.tensor_tensor(vm, vT[:], MF, AL.mult)
            C = pool.tile([P, 2 * W], f32)
            nc.vector.tensor_tensor(C, NB[:], MF2[:], AL.mult)

            # distance scans: state = min(1 + state, c); halo makes them carry-free
            LS = pool.tile([P, W], f32)
            nc.vector.tensor_tensor_scan(LS, ones52[:], C[:, 0:W], BIG, AL.add, AL.min)
            RS = pool.tile([P, W], f32)
            nc.vector.tensor_tensor_scan(RS[:, W - 1::-1], ones52[:],
                                         C[:, 2 * W - 1:W - 1:-1], BIG, AL.add, AL.min)

            # cumsum of v*m; cross-partition carry via triangular matmul
            pc = pool.tile([P, F], f32)
            nc.vector.tensor_tensor_scan(pc, vm[:], vm[:], 0.0, AL.add, AL.bypass)
            carryC = psum.tile([P, 1], f32)
            nc.tensor.matmul(carryC, LT[:], pc[:, F - 1:F], start=True, stop=True)

            D = pool.tile([P, F], f32)
            nc.vector.scalar_tensor_tensor(D, LS[:, H:W], 20.0, RS[:, 0:F], AL.min, AL.min)
            R3 = pool.tile([P, 4], f32)
            i_maxd = nc.vector.tensor_reduce(R3[:, 2:3], D[:], AX.X, AL.max)
            carryCS = pool.tile([P, 1], f32)
            i_ccs = nc.vector.tensor_copy(carryCS, carryC[:])
            tile.add_dep_helper(i_ccs.ins, i_maxd.ins, reason="order carryCS after maxD")

            # |pred - D| * m and row reductions
            d1 = pool.tile([P, F], f32)
            nc.vector.scalar_tensor_tensor(d1, pc[:], carryCS[:], D[:], AL.add, AL.subtract)
            d2 = pool.tile([P, F], f32)
            nc.vector.tensor_tensor(d2, d1[:], MF, AL.mult)
            nc.vector.tensor_reduce(R3[:, 0:1], d2[:], AX.X, AL.add, apply_absolute_value=True)
            nc.vector.tensor_reduce(R3[:, 1:2], MF, AX.X, AL.add)

            # partition reductions via PE
            rmT = psum.tile([1, P], f32)
            nc.tensor.matmul(rmT, R3[:, 2:3], I128[:], start=True, stop=True)
            sums = psum.tile([1, 2], f32)
            nc.tensor.matmul(sums, ones128[:], R3[:, 0:2], start=True, stop=True)

            # final scalars on partition 0
            G = pool.tile([1, 4], f32)
            nc.vector.tensor_reduce(G[0:1, 0:1], rmT[0:1, :], AX.X, AL.max)
            nc.vector.tensor_scalar(G[0:1, 1:2], G[0:1, 0:1], 1e-6, None, AL.add)
            nc.vector.reciprocal(G[0:1, 2:3], G[0:1, 1:2])
            OUTt = pool.tile([1, 2], f32)
            nc.vector.tensor_tensor(OUTt[0:1, 0:1], sums[0:1, 0:1], G[0:1, 2:3], AL.mult)
            nc.vector.tensor_copy(OUTt[0:1, 1:2], sums[0:1, 1:2])
            nc.sync.dma_start(out_ext.ap(), OUTt[:])

    _hoist_prologue(nc)
    nc.compile()
    return nc


def _hoist_prologue(nc):
    """Move the input DMAs and dependency-free constant generation from the
    tile-context body into `main`, ahead of the entry all-engine barrier.
    The DMA transfers + completion latency (~2.1us) and the iota/compare
    constants then overlap the fixed NEFF startup instead of following it.
    Consumer waits (S[DMAHW*] >= 16 etc.) stay where Tile placed them."""
    main_bb = nc.main_func.blocks[0]
    body_bb = nc.main_func.blocks[1]
    moved = []
    n_dma = n_memset = n_tsp = 0
    for inst in list(body_bb.instructions):
        cls = inst.__class__.__name__
        if cls == "InstDMACopy" and n_dma < 2:
            n_dma += 1
            moved.append(inst)
        elif cls in ("InstPseudoReloadLibraryIndex", "InstIota"):
            moved.append(inst)
        elif cls == "InstMemset" and n_memset < 2:
            n_memset += 1
            moved.append(inst)
        elif cls == "InstTensorScalarPtr" and n_tsp < 2:
            # the first two TSPs in the body are the LT / I128 iota-compares
            n_tsp += 1
            moved.append(inst)
    assert n_dma == 2 and n_memset == 2 and n_tsp == 2, [i.name for i in moved]
    for inst in moved:
        body_bb.instructions.remove(inst)
    for pos, inst in enumerate(moved):
        main_bb.instructions.insert(1 + pos, inst)


def _halo_views_u8(b, m):
    """b, m: [4096] bool -> [128, 208] uint8 = [b_L | b_R | m_L | m_R]."""
    b_ext_l = np.concatenate([np.zeros(H - 1, bool), [True], b])
    m_ext_l = np.concatenate([np.ones(H, bool), m])
    b_ext_r = np.concatenate([b, [True], np.zeros(H - 1, bool)])
    m_ext_r = np.concatenate([m, np.ones(H, bool)])
    idx = np.arange(P)[:, None] * F + np.arange(W)[None, :]
    return np.ascontiguousarray(np.concatenate(
        [b_ext_l[idx], b_ext_r[idx], m_ext_l[idx], m_ext_r[idx]], axis=1).astype(np.uint8))


def kernel(velocities, boundaries, mask):
    global _NC
    velocities = np.asarray(velocities, dtype=np.float32)
    boundaries = np.asarray(boundaries).astype(bool)
    mask = np.asarray(mask).astype(bool)
    assert velocities.shape == (N_CORES, P * F)

    if _NC is None:
        _NC = _build()

    in_maps = []
    for r in range(N_CORES):
        in_maps.append({
            "v": np.ascontiguousarray(velocities[r].reshape(P, F)),
            "bm": _halo_views_u8(boundaries[r], mask[r]),
        })
    res = run_bass_kernel_spmd(_NC, in_maps, list(range(N_CORES)), trace=False)
    num = sum(float(r["out"][0, 0]) for r in res.results)
    den = sum(float(r["out"][0, 1]) for r in res.results)
    return np.asarray(np.float32(num / (den + 1e-6)))


# revision 4
# speedup vs baseline: 1.0157x; 1.0157x over previous
"""Trainium2 Bass kernel for nn_ApproachingMomentumLoss (8 NeuronCores, data parallel).

Math: the reference clamps gt_distance at RADIUS=20 == DECAY_START, so momentum
is identically 1.0 in the forward pass and the loss reduces to
    loss = sum_r sum_i |cumsum(v*m)_ri - D_ri| * m_ri / (max_i D_ri + 1e-6)
           / (sum(m) + 1e-6)
with D = min(distance to nearest (boundary | ~mask), 20), virtual boundaries at
-1 and T.

Distribution: pure data parallel, one row of T=4096 per core as [128 x 32]
(element i = p*32 + f). The 20-clamp makes the distance transform local, so the
host ships a 20-halo extended uint8 view and each partition computes its
distances with two tensor_tensor_scan min-plus scans and NO cross-partition
carries. The only cross-partition work is the cumsum carry (strict-lower
triangular matmul) and the final reductions (matmuls with ones / identity).
Each core returns partial [loss_row/scale_row, mask_sum]; the unshard step sums
the 8 partials and divides.
"""
import numpy as np
import concourse.bass as bass
import concourse.bacc as bacc
import concourse.mybir as mybir
import concourse.tile as tile
from concourse.bass_utils import run_bass_kernel_spmd

f32 = mybir.dt.float32
f32r = mybir.dt.float32r
i32 = mybir.dt.int32
u8 = mybir.dt.uint8
bf16 = mybir.dt.bfloat16
AL = mybir.AluOpType
AF = mybir.ActivationFunctionType
AX = mybir.AxisListType

BIG = 1.0e9
N_CORES = 8
P, F, H = 128, 32, 20
W = F + H  # 52



def _trimmed_drain_and_barrier(self, tick_clock, wait_clock):
    """Tile's stock exit is drain + barrier + sem-clears + barrier. The second
    barrier only orders sem-clears against a subsequent execution's first user
    sem op; the next execution begins with an all-engine barrier of its own, so
    it is redundant — drop it."""
    from concourse.vector_clock import ScopedClock

    drain_inst = self.nc.sync.drain()
    wait_clock.add_sem_waits(
        drain_inst.ins, ScopedClock({None: tick_clock.global_clock})
    )
    self.nc.all_engine_barrier()
    popped = self.nc._tile_sem_poison_stack.pop()
    assert popped is self._sem_poison
    self.nc.clear_and_free_semaphores(list(self.sems.allocated().values()))
    self.nc.all_engine_barrier(sem_only=True)


def _build():
    tile.TileContext._drain_and_barrier = _trimmed_drain_and_barrier
    nc = bacc.Bacc("TRN2", target_bir_lowering=False, debug=False, num_devices=N_CORES)
    bm_ext = nc.dram_tensor("bm", [P, 2 * (F + 2 * H)], u8, kind="ExternalInput")
    v_ext = nc.dram_tensor("v", [P, F], f32, kind="ExternalInput")
    out_ext = nc.dram_tensor("out", [1, 2], f32, kind="ExternalOutput")

    with tile.TileContext(nc) as tc:
        with (
            tc.tile_pool(name="sb", bufs=1) as pool,
            tc.tile_pool(name="ps", bufs=1, space="PSUM") as psum,
        ):
            # ---- inputs (two sequencers -> parallel HWDGE queues)
            X = F + 2 * H  # 72: columns p*32-20 .. p*32+51 of the padded row
            bmT = pool.tile([P, 2 * X], u8)
            nc.sync.dma_start(bmT[:], bm_ext.ap())
            vT = pool.tile([P, F], f32)
            nc.scalar.dma_start(vT[:], v_ext.ap())

            # ---- constants. The [128,128] matmul constants are generated fully
            # on GPSIMD in the body (parallel with the DVE chain; only the PE
            # matmuls consume them). The DVE memsets get hoisted pre-barrier.
            ones52 = pool.tile([P, W], f32)
            nc.vector.memset(ones52, 1.0)
            ones128 = pool.tile([P, 1], f32)
            nc.vector.memset(ones128, 1.0)
            io128 = pool.tile([P, P], i32)
            nc.gpsimd.iota(io128, pattern=[[1, P]], base=0, channel_multiplier=-1)  # c - p
            LT = pool.tile([P, P], f32)
            nc.vector.tensor_scalar(LT, io128, 0, None, AL.is_gt)       # strict lower (as lhsT)
            I128bf = pool.tile([P, P], bf16)
            nc.vector.tensor_scalar(I128bf, io128, 0, None, AL.is_equal)

            # ---- cost tile: C = BIG * (m AND NOT b) over the extended view
            Q = pool.tile([P, X], f32)
            nc.vector.tensor_tensor(Q, bmT[:, X:2 * X], bmT[:, 0:X], AL.is_gt)  # m > b
            C = pool.tile([P, X], f32)
            nc.vector.tensor_scalar(C, Q[:], BIG, None, AL.mult)
            MF2 = pool.tile([P, F], f32)
            nc.vector.tensor_scalar(MF2, bmT[:, X + H:X + H + F], 1, None, AL.mult)  # raw row mask
            MF = MF2[:, 0:F]
            vm = pool.tile([P, F], f32)
            nc.vector.tensor_tensor(vm, vT[:], MF, AL.mult)

            # ---- distance scans over overlapping slices (carry-free via halo)
            LS = pool.tile([P, W], f32)
            nc.vector.tensor_tensor_scan(LS, ones52[:], C[:, 0:W], BIG, AL.add, AL.min)
            RS = pool.tile([P, W], f32)
            nc.vector.tensor_tensor_scan(RS[:, W - 1::-1], ones52[:], C[:, X - 1:H - 1:-1],
                                         BIG, AL.add, AL.min)

            # ---- cumsum of v*m
            pc = pool.tile([P, F], f32)
            nc.vector.tensor_tensor_scan(pc, vm[:], vm[:], 0.0, AL.add, AL.bypass)
            carryC = psum.tile([P, 1], f32)
            nc.tensor.matmul(carryC, LT[:], pc[:, F - 1:F], start=True, stop=True)

            D = pool.tile([P, F], f32)
            nc.vector.scalar_tensor_tensor(D, LS[:, H:W], 20.0, RS[:, 0:F], AL.min, AL.min)
            R3 = pool.tile([P, 4], f32)
            DMX = pool.tile([P, 1], bf16)
            i_maxd = nc.vector.tensor_reduce(DMX[:], D[:], AX.X, AL.max)               # row max D (bf16-exact)
            carryCS = pool.tile([P, 1], f32)
            i_ccs = nc.vector.tensor_copy(carryCS, carryC[:])
            # keep DVE from idling on the carry matmul: D/maxD first, then the copy
            tile.add_dep_helper(i_ccs.ins, i_maxd.ins, reason="order carryCS after maxD")

            # ---- |pred - D| * m and row reductions
            d1 = pool.tile([P, F], f32)
            nc.vector.scalar_tensor_tensor(d1, pc[:], carryCS[:], D[:], AL.add, AL.subtract)
            d2 = pool.tile([P, F], f32)
            nc.vector.tensor_tensor(d2, d1[:], MF, AL.mult)
            nc.vector.tensor_reduce(R3[:, 0:1], d2[:], AX.X, AL.add, apply_absolute_value=True)
            nc.vector.tensor_reduce(R3[:, 1:2], MF, AX.X, AL.add)                      # sum m

            # ---- partition reductions via PE
            rmT = psum.tile([1, P], f32)
            nc.tensor.matmul(rmT, DMX[:], I128bf[:], start=True, stop=True)          # transpose row maxes
            sums = psum.tile([1, 2], f32)
            nc.tensor.matmul(sums, ones128[:], R3[:, 0:2], start=True, stop=True)

            # ---- final scalars on partition 0
            G = pool.tile([1, 4], f32)
            nc.vector.tensor_reduce(G[0:1, 0:1], rmT[0:1, :], AX.X, AL.max)            # global max D
            nc.vector.tensor_scalar(G[0:1, 1:2], G[0:1, 0:1], 1e-6, None, AL.add)
            nc.vector.reciprocal(G[0:1, 2:3], G[0:1, 1:2])
            OUTt = pool.tile([1, 2], f32)
            nc.vector.tensor_tensor(OUTt[0:1, 0:1], sums[0:1, 0:1], G[0:1, 2:3], AL.mult)
            nc.vector.tensor_copy(OUTt[0:1, 1:2], sums[0:1, 1:2])
            nc.sync.dma_start(out_ext.ap(), OUTt[:])

    _hoist_input_dmas(nc)
    nc.compile()
    return nc


def _hoist_input_dmas(nc):
    """Move the two input DMACopy instructions from the tile-context body into
    `main`, ahead of the entry all-engine barrier. Their transfers + completion
    latency (~2.1us) then overlap the fixed NEFF prologue instead of following
    it. Consumer waits (S[DMAHW*] >= 16) stay where Tile placed them."""
    main_bb = nc.main_func.blocks[0]
    body_bb = nc.main_func.blocks[1]
    moved = []
    n_dma = n_memset = 0
    for inst in list(body_bb.instructions):
        cls = inst.__class__.__name__
        if cls == "InstDMACopy" and n_dma < 2:
            n_dma += 1
            moved.append(inst)
        elif cls in ("InstPseudoReloadLibraryIndex", "InstIota"):
            moved.append(inst)
        elif cls == "InstMemset" and n_memset < 2:
            n_memset += 1
            moved.append(inst)
        elif cls == "InstTensorScalarPtr" and n_tsp < 2:
            # first two TSPs are the LT / I128 iota-compares
            n_tsp += 1
            moved.append(inst)
    assert n_dma == 2 and n_memset == 2 and n_tsp == 2, [i.name for i in moved]
    for inst in moved:
        body_bb.instructions.remove(inst)
    for pos, inst in enumerate(moved):
        main_bb.instructions.insert(1 + pos, inst)


def halo_views_u8(b, m):
    """b, m: [4096] bool -> bmx [128, 144] uint8 = [bx(72) | mx(72)], where
    column j of partition p is padded-row element p*32 - 20 + j."""
    b_ext = np.concatenate([np.zeros(H - 1, bool), [True], b, [True], np.zeros(H - 1, bool)])
    m_ext = np.concatenate([np.ones(H, bool), m, np.ones(H, bool)])
    idx = np.arange(P)[:, None] * F + np.arange(F + 2 * H)[None, :]
    return np.ascontiguousarray(np.concatenate(
        [b_ext[idx], m_ext[idx]], axis=1).astype(np.uint8))




_NC = None


def kernel(velocities, boundaries, mask):
    global _NC
    velocities = np.asarray(velocities, dtype=np.float32)
    boundaries = np.asarray(boundaries).astype(bool)
    mask = np.asarray(mask).astype(bool)
    assert velocities.shape == (N_CORES, P * F)

    if _NC is None:
        _NC = _build()

    in_maps = []
    for r in range(N_CORES):
        in_maps.append({
            "v": np.ascontiguousarray(velocities[r].reshape(P, F)),
            "bm": halo_views_u8(boundaries[r], mask[r]),
        })
    res = run_bass_kernel_spmd(_NC, in_maps, list(range(N_CORES)), trace=False)
    num = sum(float(r["out"][0, 0]) for r in res.results)
    den = sum(float(r["out"][0, 1]) for r in res.results)
    return np.asarray(np.float32(num / (den + 1e-6)))
